# revision 29
# baseline (speedup 1.0000x reference)
"""Multi-head attention kernel for Trainium2 (8 NeuronCores).

Problem: B=4, T=2048, U=1024, H=16 heads, D=64. Full (non-causal) softmax
attention per head. 64 independent (head, batch) problems.

Sharding: core c owns batch b = c//2 and head block hb = c%2 (8 contiguous
heads = 512 contiguous channels). No cross-core communication.

Per-core algorithm (T=2048, DD=512 channels, 8 local heads of D=64):
  - Q, K: loaded fp32, converted to bf16 on DVE, written to a bf16 DRAM
    scratch, then transpose-loaded (DMA xbar) into QT/KT [d, t] tiles
    (two heads per 128-partition tile).
  - V is converted to bf16 (DVE) into a per-t-chunk [128, 8*65] layout where
    each head's 64 columns are augmented with a ones column (computes the
    softmax denominator for free in the second matmul).
  - For each head h, q-half qh (1024 q), the 16 k-chunks are processed as
    beats through a ring of 3 PSUM score tiles [128,1024] (beat_mode="ring3",
    the HW-measured winner; "ab" = asymmetric [128,2048]+[128,1024] batching
    is kept as an option):
      mm1: scoresT[k, q] = KT_chunk.T @ QT  (fp32 PSUM)
      exp: probsT = exp(scoresT / 8)        (one ACT call per beat, bf16)
      mm2: outT[65, q] += V_aug[kc].T @ probsT  (PSUM accumulate, fp32)
    The ACT engine (exp @ 1 elem/lane/cycle) is the kernel bottleneck.
    mm2 is software-pipelined two beats behind mm1/exp so the in-order PE
    queue never stalls in mm2 waiting for its own beat's exp, and the
    3-deep ring hides the exp->mm1 WAR semaphore latency.
    Then: evacuate outT to SBUF; r = 1/outT[64]; partition-broadcast r
    (GpSimd, the only library op); out = outT[0:64] * r; split fp32 into
    bf16 hi+lo (DVE); DMA-transpose both to [q, d] orientation; DVE re-add
    to fp32; store (GpSimd-triggered DMA).
"""

import os
import sys

sys.path.insert(0, "/opt/trn_rl_repo")

import ml_dtypes
import numpy as np

import concourse.bass as bass
import concourse.bacc as bacc
import concourse.mybir as mybir
import concourse.tile as tile
from concourse import library_config
from concourse.bass_utils import run_bass_kernel_spmd

F32 = mybir.dt.float32
BF16 = mybir.dt.bfloat16
EXP = mybir.ActivationFunctionType.Exp

B, T, U = 4, 2048, 1024
H_TOTAL, D = 16, 64
DD = 512          # channels per core (8 heads)
H = 8             # heads per core
NQ = 1024         # q-half size
KC = 16           # k chunks of 128
TC = 16           # t chunks of 128
HP = 4            # head pairs
N_CORES = 8
SCALE = 1.0 / 8.0  # 1/sqrt(D)

# Beat schedules over the 16 k-chunks: strictly alternate the A ([128,2048],
# up to 2 chunks) and B ([128,1024], 1 chunk) PSUM score tiles, with the
# parity flipped on odd (h,qh) iterations so consecutive iterations also
# alternate (avoids mm1-waits-for-exp WAR stalls at iteration boundaries).
BEATS_EVEN = [("A", 2), ("B", 1), ("A", 2), ("B", 1), ("A", 2), ("B", 1),
              ("A", 2), ("B", 1), ("A", 2), ("B", 1), ("A", 1)]
BEATS_ODD = [("B", 1), ("A", 2), ("B", 1), ("A", 2), ("B", 1), ("A", 2),
             ("B", 1), ("A", 2), ("B", 1), ("A", 2), ("B", 1)]
assert sum(w for _, w in BEATS_EVEN) == KC
assert sum(w for _, w in BEATS_ODD) == KC


def build_program(nc, bench_iters=0, stages=("pre", "mm1", "exp", "mm2", "norm"),
                  depth=0, beat_mode="ring3"):
    if bench_iters:
        # Timing-only variant: big tensors are Internal (values irrelevant),
        # external I/O is tiny, and the whole body runs in a For_i loop.
        in_flag = nc.dram_tensor("in_flag", [1, 1], F32, kind="ExternalInput").ap()
        out_flag = nc.dram_tensor("out_flag", [1, 1], F32, kind="ExternalOutput").ap()
        q_d = nc.dram_tensor("querys", [T, DD], F32).ap()
        k_d = nc.dram_tensor("keys", [T, DD], F32).ap()
        v_d = nc.dram_tensor("values", [T, DD], F32).ap()
        o_d = nc.dram_tensor("out", [T, DD], F32).ap()
    else:
        q_d = nc.dram_tensor("querys", [T, DD], F32, kind="ExternalInput").ap()
        k_d = nc.dram_tensor("keys", [T, DD], F32, kind="ExternalInput").ap()
        v_d = nc.dram_tensor("values", [T, DD], F32, kind="ExternalInput").ap()
        o_d = nc.dram_tensor("out", [T, DD], F32, kind="ExternalOutput").ap()
    # per-head-pair scratch tensors: transposes of one head pair only
    # depend on that pair's stores (no false whole-tensor serialization)
    qbf_d = [nc.dram_tensor(f"qbf_scratch{i}", [T, 128], BF16).ap()
             for i in range(HP)]
    kbf_d = [nc.dram_tensor(f"kbf_scratch{i}", [T, 128], BF16).ap()
             for i in range(HP)]

    import contextlib

    with tile.TileContext(nc) as tc:
        if bench_iters:
            nc.sync.dma_start(o_d[0:1, 0:1], in_flag[:])  # consume input
            loop_cm = tc.For_i(0, bench_iters, 1)
        else:
            loop_cm = contextlib.nullcontext()
        with (
            tc.tile_pool(name="persist", bufs=1) as persist,
            tc.tile_pool(name="stage", bufs=4) as stage,
            tc.tile_pool(name="probs", bufs=3) as probs_pool,
            tc.tile_pool(name="norm", bufs=2) as norm_pool,
            tc.tile_pool(name="ps_a", bufs=1, space=bass.MemorySpace.PSUM) as ps_a,
            tc.tile_pool(name="ps_b",
                         bufs=(3 if beat_mode == "ring3" else 1),
                         space=bass.MemorySpace.PSUM) as ps_b,
            tc.tile_pool(name="ps", bufs=1, space=bass.MemorySpace.PSUM) as ps,
        ):
            # ---- once-only setup (outside the bench For_i loop) ----
            # GpSimd library (for partition_broadcast) loads while DVE does
            # the bf16 conversions; exp ACT-table load is hoisted via a
            # dummy activation.
            nc.gpsimd.load_library(library_config.attn)
            warm = persist.tile([1, 1], F32, tag="warm")
            nc.vector.memset(warm[:], 0.0)
            warm_o = persist.tile([1, 1], F32, tag="warm_o")
            nc.scalar.activation(warm_o[:], warm[:], EXP)

            # persistent tiles
            vc = [
                persist.tile([128, H * 65], BF16, tag=f"vc{c}", name=f"vc{c}")
                for c in range(TC)
            ]
            for c in range(TC):
                nc.gpsimd.memset(vc[c][:], 1.0)
            qt = [
                persist.tile([128, T], BF16, tag=f"qt{hp}", name=f"qt{hp}")
                for hp in range(HP)
            ]
            kt = [
                persist.tile([128, T], BF16, tag=f"kt{hp}", name=f"kt{hp}")
                for hp in range(HP)
            ]

            if "pre" not in stages:
                # ablation benches without the preamble: give qt/kt defined
                # contents so mm1 can run standalone
                for t_ in qt + kt:
                    nc.vector.memset(t_[:], 0.0)

            v_3d = v_d.rearrange("(c p) d -> c p d", p=128)
            q3s = q_d.rearrange("(c p) d -> c p d", p=128)
            q3d = [t.rearrange("(c p) d -> c p d", p=128) for t in qbf_d]
            k3s = k_d.rearrange("(c p) d -> c p d", p=128)
            k3d = [t.rearrange("(c p) d -> c p d", p=128) for t in kbf_d]

            with loop_cm:

                def qk_group(src3, dst3, g, which):
                    # one DMA + one convert for 4 t-chunks; one store per
                    # head pair into its own scratch tensor
                    s = stage.tile([128, 4, DD], F32, tag="qkstage",
                                   name=f"{which}s{g}")
                    nc.sync.dma_start(s[:], src3[4 * g : 4 * g + 4].rearrange(
                        "c p d -> p c d"))
                    sb = stage.tile([128, 4, DD], BF16, tag="qkbf", bufs=3,
                                    name=f"{which}b{g}")
                    nc.vector.tensor_copy(sb[:], s[:])
                    sb4 = sb[:].rearrange("p c (i d) -> p c i d", i=HP)
                    for i in range(HP):
                        nc.sync.dma_start(
                            dst3[i][4 * g : 4 * g + 4].rearrange("c p d -> p c d"),
                            sb4[:, :, i],
                        )

                def v_group(g):
                    vs = stage.tile([128, 4, DD], F32, tag="vstage", name=f"vs{g}")
                    nc.sync.dma_start(vs[:], v_3d[4 * g : 4 * g + 4].rearrange(
                        "c p d -> p c d"))
                    for i in range(4):
                        nc.gpsimd.tensor_copy(
                            vc[4 * g + i][:].rearrange(
                                "p (h e) -> p h e", e=65)[:, :, 0:64],
                            vs[:, i].rearrange("p (h e) -> p h e", e=64),
                        )

                def xbar_tpose(which, hp, th):
                    # transpose-load one t-half of one tensor from bf16 scratch
                    src, dst = (kbf_d, kt) if which == "k" else (qbf_d, qt)
                    tsl = slice(th * 1024, (th + 1) * 1024)
                    nc.sync.dma_start(dst[hp][:, tsl], src[hp][tsl, :],
                                      transpose=True)

                # preamble: full-width grouped chunk loads (4 t-chunks per
                # DMA); first t-half of K and Q converts first so head-pair-0
                # transposes land fast, then K's second half (the kc loop
                # needs all of kt0 before qt0's second half is touched).
                def run_preamble():
                    do_tp = "notp" not in stages
                    qk_group(k3s, k3d, 0, "k")
                    qk_group(q3s, q3d, 0, "q")
                    qk_group(k3s, k3d, 1, "k")
                    qk_group(q3s, q3d, 1, "q")
                    if do_tp:
                        xbar_tpose("k", 0, 0)
                        xbar_tpose("q", 0, 0)
                    v_group(0)
                    qk_group(k3s, k3d, 2, "k")
                    qk_group(k3s, k3d, 3, "k")
                    if do_tp:
                        xbar_tpose("k", 0, 1)
                    v_group(1)
                    qk_group(q3s, q3d, 2, "q")
                    qk_group(q3s, q3d, 3, "q")
                    if do_tp:
                        xbar_tpose("q", 0, 1)
                    v_group(2)
                    v_group(3)
                    if do_tp:
                        for th in range(2):
                            xbar_tpose("k", 1, th)
                            xbar_tpose("q", 1, th)

                if "pre" in stages:
                    run_preamble()

                # ---- main loop ----
                # norm emission is deferred by one (h, qh) iteration so the
                # outp PSUM evacuation copy is first in the DVE queue after
                # mm2 (the heavy norm chain of the previous iteration would
                # otherwise delay it and stall the next iteration's mm2).
                pended_norm = []

                def norm_block(h, qh, outsb):
                    r = norm_pool.tile([1, NQ], F32, tag="r", name="r")
                    nc.vector.reciprocal(r[:], outsb[64:65, :])
                    bc = norm_pool.tile([64, NQ], F32, tag="bc", name="bc")
                    nc.gpsimd.partition_broadcast(bc[:], r[:])
                    ob = norm_pool.tile([64, NQ], F32, tag="ob", name="ob")
                    nc.vector.tensor_mul(ob[:], outsb[0:64, :], bc[:])
                    # split into bf16 hi+lo halves of one tile so a single
                    # xbar transpose moves both to [q, d] orientation
                    hilo = norm_pool.tile([64, 2 * NQ], BF16, tag="hilo", name="hilo")
                    nc.vector.tensor_copy(hilo[:, 0:NQ], ob[:])
                    nc.vector.tensor_sub(hilo[:, NQ : 2 * NQ], ob[:], hilo[:, 0:NQ])
                    # hilo_t[p, m, l] = hilo[l, m*128+p]: m 0..7 = hi, 8..15 = lo
                    hilo_t = norm_pool.tile([128, 16 * 64], BF16, tag="hilo_t",
                                            name="hilo_t")
                    nc.sync.dma_start(
                        hilo_t[:].rearrange("p (m l) -> p m l", l=64),
                        hilo[:],
                        transpose=True,
                    )
                    # hi occupies columns 0:512 (m 0..7), lo columns 512:1024
                    ob2 = norm_pool.tile([128, 8 * 64], F32, tag="ob2", name="ob2")
                    nc.vector.tensor_add(
                        ob2[:], hilo_t[:, 0:512], hilo_t[:, 512:1024]
                    )
                    # out[qh*1024 + m*128 + p, h*64 + d] <- ob2[p, m*64+d]
                    dest = o_d[
                        qh * NQ : (qh + 1) * NQ, h * 64 : (h + 1) * 64
                    ].rearrange("(m p) d -> p m d", p=128)
                    nc.gpsimd.dma_start(
                        dest, ob2[:].rearrange("p (m l) -> p m l", l=64)
                    )

                for h in range(H):
                    hp, base = h // 2, (h % 2) * 64
                    if h == 1 and "pre" in stages and "notp" not in stages:
                        for th in range(2):
                            xbar_tpose("k", 2, th)
                            xbar_tpose("q", 2, th)
                    if h == 3 and "pre" in stages and "notp" not in stages:
                        for th in range(2):
                            xbar_tpose("k", 3, th)
                            xbar_tpose("q", 3, th)
                    for qh in range(2):
                        outp = ps.tile([65, NQ], F32, tag="outp", name="outp")
                        if beat_mode == "ring3":
                            beats = [("B", 1)] * KC
                        else:
                            beats = (BEATS_EVEN if (2 * h + qh) % 2 == 0
                                     else BEATS_ODD)

                        def emit_mm2(kcs, pb):
                            for i, kci in enumerate(kcs):
                                vsl = vc[kci][:, h * 65 : (h + 1) * 65]
                                for j in range(2):
                                    nc.tensor.matmul(
                                        outp[:, j * 512 : (j + 1) * 512],
                                        vsl,
                                        pb[:, i * NQ + j * 512 : i * NQ + (j + 1) * 512],
                                        start=(kci == 0),
                                        stop=(kci == KC - 1),
                                    )

                        # software pipeline: mm2 trails mm1/exp by two beats,
                        # so the PE FIFO always has the next beat's mm1 ready
                        # to run during an exp (never stalls in mm2 waiting
                        # for the exp of its own beat).
                        pend_mm2 = []
                        kc = 0
                        for buf, wide in beats:
                            if buf == "A":
                                sc = ps_a.tile([128, 2 * NQ], F32, tag="sa", name="sa")
                            else:
                                sc = ps_b.tile([128, NQ], F32, tag="sb", name="sb")
                            n = wide * NQ
                            kcs = list(range(kc, min(kc + wide, KC)))
                            kc += wide
                            if "mm1" in stages:
                                for i, kci in enumerate(kcs):
                                    lhsT = kt[hp][
                                        base : base + 64, kci * 128 : (kci + 1) * 128
                                    ]
                                    for j in range(2):
                                        nc.tensor.matmul(
                                            sc[:, i * NQ + j * 512 : i * NQ + (j + 1) * 512],
                                            lhsT,
                                            qt[hp][
                                                base : base + 64,
                                                qh * NQ + j * 512 : qh * NQ
                                                + (j + 1) * 512,
                                            ],
                                            start=True,
                                            stop=True,
                                        )
                            if "exp" not in stages:
                                continue
                            pb = probs_pool.tile(
                                [128, n], BF16, tag=f"pb{wide}", bufs=4,
                                name="pb"
                            )
                            nc.scalar.activation(pb[:], sc[:, 0:n], EXP, scale=SCALE)
                            if "mm2" not in stages:
                                continue
                            pend_mm2.append((kcs, pb))
                            if len(pend_mm2) > 2:
                                emit_mm2(*pend_mm2.pop(0))
                        for args in pend_mm2:
                            emit_mm2(*args)
                        if "mm2" not in stages or "norm" not in stages:
                            continue
                        # evacuate outT from PSUM quickly (frees outp for the
                        # next iteration's mm2), then run the deferred norm.
                        outsb = norm_pool.tile([65, NQ], F32, tag="outsb", name="outsb")
                        nc.vector.tensor_copy(outsb[:], outp[:])
                        for args in pended_norm:
                            norm_block(*args)
                        pended_norm = [(h, qh, outsb)]
                        if h == H - 1 and qh == 1:
                            # last iteration: run its norm eagerly (nothing
                            # left to overlap with; shortens the tail)
                            for args in pended_norm:
                                norm_block(*args)
                            pended_norm = []
        if bench_iters:
            nc.sync.dma_start(out_flag[:], o_d[0:1, 0:1])
    return nc


_CACHED = None


def _get_program():
    global _CACHED
    if _CACHED is None:
        nc = bacc.Bacc("TRN2", target_bir_lowering=False, debug=False)
        _CACHED = build_program(nc)
        _CACHED.compile()
    return _CACHED


def _make_in_maps(querys, keys, values):
    querys = np.ascontiguousarray(np.asarray(querys, dtype=np.float32))
    keys = np.ascontiguousarray(np.asarray(keys, dtype=np.float32))
    values = np.ascontiguousarray(np.asarray(values, dtype=np.float32))
    in_maps = []
    for c in range(N_CORES):
        b, hb = c // 2, c % 2
        sl = slice(hb * DD, (hb + 1) * DD)
        in_maps.append(
            {
                "querys": querys[b, :, sl],
                "keys": keys[b, :, sl],
                "values": values[b, :, sl],
            }
        )
    return in_maps


def kernel(querys, keys, values):
    nc = _get_program()
    in_maps = _make_in_maps(querys, keys, values)
    res = run_bass_kernel_spmd(nc, in_maps, list(range(N_CORES)))
    out = np.empty((B, T, U), dtype=np.float32)
    for c in range(N_CORES):
        b, hb = c // 2, c % 2
        out[b, :, hb * DD : (hb + 1) * DD] = res.results[c]["out"]
    return out


# revision 30
# speedup vs baseline: 1.0740x; 1.0740x over previous
"""Multi-head attention kernel for Trainium2 (8 NeuronCores).

Problem: B=4, T=2048, U=1024, H=16 heads, D=64. Full (non-causal) softmax
attention per head. 64 independent (head, batch) problems.

Sharding: core c owns batch b = c//2 and head block hb = c%2 (8 contiguous
heads = 512 contiguous channels). No cross-core communication.

Per-core algorithm (T=2048, DD=512 channels, 8 local heads of D=64):
  - Q, K: loaded fp32, converted to bf16 on DVE, written to a bf16 DRAM
    scratch, then transpose-loaded (DMA xbar) into QT/KT [d, t] tiles
    (two heads per 128-partition tile).
  - V is converted to bf16 (DVE) into a per-t-chunk [128, 8*65] layout where
    each head's 64 columns are augmented with a ones column (computes the
    softmax denominator for free in the second matmul).
  - For each head h, q-half qh (1024 q), the 16 k-chunks are processed as
    beats through a ring of 3 PSUM score tiles [128,1024] (beat_mode="ring3",
    the HW-measured winner; "ab" = asymmetric [128,2048]+[128,1024] batching
    is kept as an option):
      mm1: scoresT[k, q] = KT_chunk.T @ QT  (fp32 PSUM)
      exp: probsT = exp(scoresT / 8)        (one ACT call per beat, bf16)
      mm2: outT[65, q] += V_aug[kc].T @ probsT  (PSUM accumulate, fp32)
    The ACT engine (exp @ 1 elem/lane/cycle) is the kernel bottleneck.
    mm2 is software-pipelined two beats behind mm1/exp so the in-order PE
    queue never stalls in mm2 waiting for its own beat's exp, and the
    3-deep ring hides the exp->mm1 WAR semaphore latency.
    Then: evacuate outT to SBUF; r = 1/outT[64]; partition-broadcast r
    (GpSimd, the only library op); out = outT[0:64] * r; split fp32 into
    bf16 hi+lo (DVE); DMA-transpose both to [q, d] orientation; DVE re-add
    to fp32; store (GpSimd-triggered DMA).
"""

import sys

sys.path.insert(0, "/opt/trn_rl_repo")

import numpy as np

import concourse.bass as bass
import concourse.bacc as bacc
import concourse.mybir as mybir
import concourse.tile as tile
from concourse import library_config
from concourse.bass_utils import run_bass_kernel_spmd

F32 = mybir.dt.float32
BF16 = mybir.dt.bfloat16
EXP = mybir.ActivationFunctionType.Exp

B, T, U = 4, 2048, 1024
H_TOTAL, D = 16, 64
DD = 512          # channels per core (8 heads)
H = 8             # heads per core
NQ = 1024         # q-half size
KC = 16           # k chunks of 128
TC = 16           # t chunks of 128
HP = 4            # head pairs
N_CORES = 8
SCALE = 1.0 / 8.0  # 1/sqrt(D)

# Beat schedules over the 16 k-chunks: strictly alternate the A ([128,2048],
# up to 2 chunks) and B ([128,1024], 1 chunk) PSUM score tiles, with the
# parity flipped on odd (h,qh) iterations so consecutive iterations also
# alternate (avoids mm1-waits-for-exp WAR stalls at iteration boundaries).
BEATS_EVEN = [("A", 2), ("B", 1), ("A", 2), ("B", 1), ("A", 2), ("B", 1),
              ("A", 2), ("B", 1), ("A", 2), ("B", 1), ("A", 1)]
BEATS_ODD = [("B", 1), ("A", 2), ("B", 1), ("A", 2), ("B", 1), ("A", 2),
             ("B", 1), ("A", 2), ("B", 1), ("A", 2), ("B", 1)]
assert sum(w for _, w in BEATS_EVEN) == KC
assert sum(w for _, w in BEATS_ODD) == KC


def build_program(nc, bench_iters=0, stages=("pre", "mm1", "exp", "mm2", "norm"),
                  depth=0, beat_mode="ring3"):
    if bench_iters:
        # Timing-only variant: big tensors are Internal (values irrelevant),
        # external I/O is tiny, and the whole body runs in a For_i loop.
        in_flag = nc.dram_tensor("in_flag", [1, 1], F32, kind="ExternalInput").ap()
        out_flag = nc.dram_tensor("out_flag", [1, 1], F32, kind="ExternalOutput").ap()
        q_d = nc.dram_tensor("querys", [T, DD], F32).ap()
        k_d = nc.dram_tensor("keys", [T, DD], F32).ap()
        v_d = nc.dram_tensor("values", [T, DD], F32).ap()
        o_d = nc.dram_tensor("out", [T, DD], F32).ap()
    else:
        q_d = nc.dram_tensor("querys", [T, DD], F32, kind="ExternalInput").ap()
        k_d = nc.dram_tensor("keys", [T, DD], F32, kind="ExternalInput").ap()
        v_d = nc.dram_tensor("values", [T, DD], F32, kind="ExternalInput").ap()
        o_d = nc.dram_tensor("out", [T, DD], F32, kind="ExternalOutput").ap()
    # per-head-pair scratch tensors: transposes of one head pair only
    # depend on that pair's stores (no false whole-tensor serialization)
    qbf_d = [nc.dram_tensor(f"qbf_scratch{i}", [T, 128], BF16).ap()
             for i in range(HP)]
    kbf_d = [nc.dram_tensor(f"kbf_scratch{i}", [T, 128], BF16).ap()
             for i in range(HP)]

    import contextlib

    with tile.TileContext(nc) as tc:
        if bench_iters:
            nc.sync.dma_start(o_d[0:1, 0:1], in_flag[:])  # consume input
            loop_cm = tc.For_i(0, bench_iters, 1)
        else:
            loop_cm = contextlib.nullcontext()
        with (
            tc.tile_pool(name="persist", bufs=1) as persist,
            tc.tile_pool(name="stage", bufs=4) as stage,
            tc.tile_pool(name="probs", bufs=3) as probs_pool,
            tc.tile_pool(name="norm", bufs=2) as norm_pool,
            tc.tile_pool(name="ps_a", bufs=1, space=bass.MemorySpace.PSUM) as ps_a,
            tc.tile_pool(name="ps_b",
                         bufs=(3 if beat_mode == "ring3" else 1),
                         space=bass.MemorySpace.PSUM) as ps_b,
            tc.tile_pool(name="ps", bufs=1, space=bass.MemorySpace.PSUM) as ps,
        ):
            # ---- once-only setup (outside the bench For_i loop) ----
            # GpSimd library (for partition_broadcast) loads while DVE does
            # the bf16 conversions; exp ACT-table load is hoisted via a
            # dummy activation.
            nc.gpsimd.load_library(library_config.attn)
            warm = persist.tile([1, 1], F32, tag="warm")
            nc.vector.memset(warm[:], 0.0)
            warm_o = persist.tile([1, 1], F32, tag="warm_o")
            nc.scalar.activation(warm_o[:], warm[:], EXP)

            # persistent tiles
            vc = [
                persist.tile([128, H * 65], BF16, tag=f"vc{c}", name=f"vc{c}")
                for c in range(TC)
            ]
            for c in range(TC):
                nc.gpsimd.memset(vc[c][:], 1.0)
            qt = [
                persist.tile([128, T], BF16, tag=f"qt{hp}", name=f"qt{hp}")
                for hp in range(HP)
            ]
            kt = [
                persist.tile([128, T], BF16, tag=f"kt{hp}", name=f"kt{hp}")
                for hp in range(HP)
            ]

            if "pre" not in stages:
                # ablation benches without the preamble: give qt/kt defined
                # contents so mm1 can run standalone
                for t_ in qt + kt:
                    nc.vector.memset(t_[:], 0.0)

            v_3d = v_d.rearrange("(c p) d -> c p d", p=128)
            q3s = q_d.rearrange("(c p) d -> c p d", p=128)
            q3d = [t.rearrange("(c p) d -> c p d", p=128) for t in qbf_d]
            k3s = k_d.rearrange("(c p) d -> c p d", p=128)
            k3d = [t.rearrange("(c p) d -> c p d", p=128) for t in kbf_d]

            with loop_cm:

                def qk_group(src3, dst3, g, which):
                    # one DMA + one convert for 4 t-chunks; one store per
                    # head pair into its own scratch tensor
                    s = stage.tile([128, 4, DD], F32, tag="qkstage",
                                   name=f"{which}s{g}")
                    nc.sync.dma_start(s[:], src3[4 * g : 4 * g + 4].rearrange(
                        "c p d -> p c d"))
                    sb = stage.tile([128, 4, DD], BF16, tag="qkbf", bufs=3,
                                    name=f"{which}b{g}")
                    nc.vector.tensor_copy(sb[:], s[:])
                    sb4 = sb[:].rearrange("p c (i d) -> p c i d", i=HP)
                    for i in range(HP):
                        nc.sync.dma_start(
                            dst3[i][4 * g : 4 * g + 4].rearrange("c p d -> p c d"),
                            sb4[:, :, i],
                        )

                def v_group(g):
                    vs = stage.tile([128, 4, DD], F32, tag="vstage", name=f"vs{g}")
                    nc.sync.dma_start(vs[:], v_3d[4 * g : 4 * g + 4].rearrange(
                        "c p d -> p c d"))
                    for i in range(4):
                        nc.gpsimd.tensor_copy(
                            vc[4 * g + i][:].rearrange(
                                "p (h e) -> p h e", e=65)[:, :, 0:64],
                            vs[:, i].rearrange("p (h e) -> p h e", e=64),
                        )

                def xbar_tpose(which, hp, th):
                    # transpose-load one t-half of one tensor from bf16 scratch
                    src, dst = (kbf_d, kt) if which == "k" else (qbf_d, qt)
                    tsl = slice(th * 1024, (th + 1) * 1024)
                    nc.sync.dma_start(dst[hp][:, tsl], src[hp][tsl, :],
                                      transpose=True)

                # preamble: full-width grouped chunk loads (4 t-chunks per
                # DMA); first t-half of K and Q converts first so head-pair-0
                # transposes land fast, then K's second half (the kc loop
                # needs all of kt0 before qt0's second half is touched).
                def run_preamble():
                    do_tp = "notp" not in stages
                    qk_group(k3s, k3d, 0, "k")
                    qk_group(q3s, q3d, 0, "q")
                    qk_group(k3s, k3d, 1, "k")
                    qk_group(q3s, q3d, 1, "q")
                    if do_tp:
                        xbar_tpose("k", 0, 0)
                        xbar_tpose("q", 0, 0)
                    v_group(0)
                    qk_group(k3s, k3d, 2, "k")
                    qk_group(k3s, k3d, 3, "k")
                    if do_tp:
                        xbar_tpose("k", 0, 1)
                    v_group(1)
                    qk_group(q3s, q3d, 2, "q")
                    qk_group(q3s, q3d, 3, "q")
                    if do_tp:
                        xbar_tpose("q", 0, 1)
                    v_group(2)
                    v_group(3)
                    if do_tp:
                        for th in range(2):
                            xbar_tpose("k", 1, th)
                            xbar_tpose("q", 1, th)

                if "pre" in stages:
                    run_preamble()

                # ---- main loop ----
                # norm emission is deferred by one (h, qh) iteration so the
                # outp PSUM evacuation copy is first in the DVE queue after
                # mm2 (the heavy norm chain of the previous iteration would
                # otherwise delay it and stall the next iteration's mm2).
                pended_norm = []

                def norm_block(h, qh, outsb):
                    r = norm_pool.tile([1, NQ], F32, tag="r", name="r")
                    nc.vector.reciprocal(r[:], outsb[64:65, :])
                    bc = norm_pool.tile([64, NQ], F32, tag="bc", name="bc")
                    nc.gpsimd.partition_broadcast(bc[:], r[:])
                    ob = norm_pool.tile([64, NQ], F32, tag="ob", name="ob")
                    nc.vector.tensor_mul(ob[:], outsb[0:64, :], bc[:])
                    # split into bf16 hi+lo halves of one tile so a single
                    # xbar transpose moves both to [q, d] orientation
                    hilo = norm_pool.tile([64, 2 * NQ], BF16, tag="hilo", name="hilo")
                    nc.vector.tensor_copy(hilo[:, 0:NQ], ob[:])
                    nc.vector.tensor_sub(hilo[:, NQ : 2 * NQ], ob[:], hilo[:, 0:NQ])
                    # hilo_t[p, m, l] = hilo[l, m*128+p]: m 0..7 = hi, 8..15 = lo
                    hilo_t = norm_pool.tile([128, 16 * 64], BF16, tag="hilo_t",
                                            name="hilo_t")
                    nc.sync.dma_start(
                        hilo_t[:].rearrange("p (m l) -> p m l", l=64),
                        hilo[:],
                        transpose=True,
                    )
                    # hi occupies columns 0:512 (m 0..7), lo columns 512:1024
                    ob2 = norm_pool.tile([128, 8 * 64], F32, tag="ob2", name="ob2")
                    nc.vector.tensor_add(
                        ob2[:], hilo_t[:, 0:512], hilo_t[:, 512:1024]
                    )
                    # out[qh*1024 + m*128 + p, h*64 + d] <- ob2[p, m*64+d]
                    dest = o_d[
                        qh * NQ : (qh + 1) * NQ, h * 64 : (h + 1) * 64
                    ].rearrange("(m p) d -> p m d", p=128)
                    nc.gpsimd.dma_start(
                        dest, ob2[:].rearrange("p (m l) -> p m l", l=64)
                    )

                for h in range(H):
                    hp, base = h // 2, (h % 2) * 64
                    if h == 1 and "pre" in stages and "notp" not in stages:
                        for th in range(2):
                            xbar_tpose("k", 2, th)
                            xbar_tpose("q", 2, th)
                    if h == 3 and "pre" in stages and "notp" not in stages:
                        for th in range(2):
                            xbar_tpose("k", 3, th)
                            xbar_tpose("q", 3, th)
                    for qh in range(2):
                        outp = ps.tile([65, NQ], F32, tag="outp", name="outp")
                        if beat_mode == "ring3":
                            beats = [("B", 1)] * KC
                        else:
                            beats = (BEATS_EVEN if (2 * h + qh) % 2 == 0
                                     else BEATS_ODD)

                        def emit_mm2(kcs, pb):
                            for i, kci in enumerate(kcs):
                                vsl = vc[kci][:, h * 65 : (h + 1) * 65]
                                for j in range(2):
                                    nc.tensor.matmul(
                                        outp[:, j * 512 : (j + 1) * 512],
                                        vsl,
                                        pb[:, i * NQ + j * 512 : i * NQ + (j + 1) * 512],
                                        start=(kci == 0),
                                        stop=(kci == KC - 1),
                                    )

                        # software pipeline: mm2 trails mm1/exp by two beats,
                        # so the PE FIFO always has the next beat's mm1 ready
                        # to run during an exp (never stalls in mm2 waiting
                        # for the exp of its own beat).
                        pend_mm2 = []
                        kc = 0
                        for buf, wide in beats:
                            if buf == "A":
                                sc = ps_a.tile([128, 2 * NQ], F32, tag="sa", name="sa")
                            else:
                                sc = ps_b.tile([128, NQ], F32, tag="sb", name="sb")
                            n = wide * NQ
                            kcs = list(range(kc, min(kc + wide, KC)))
                            kc += wide
                            if "mm1" in stages:
                                for i, kci in enumerate(kcs):
                                    lhsT = kt[hp][
                                        base : base + 64, kci * 128 : (kci + 1) * 128
                                    ]
                                    for j in range(2):
                                        nc.tensor.matmul(
                                            sc[:, i * NQ + j * 512 : i * NQ + (j + 1) * 512],
                                            lhsT,
                                            qt[hp][
                                                base : base + 64,
                                                qh * NQ + j * 512 : qh * NQ
                                                + (j + 1) * 512,
                                            ],
                                            start=True,
                                            stop=True,
                                        )
                            if "exp" not in stages:
                                continue
                            pb = probs_pool.tile(
                                [128, n], BF16, tag=f"pb{wide}", bufs=4,
                                name="pb"
                            )
                            nc.scalar.activation(pb[:], sc[:, 0:n], EXP, scale=SCALE)
                            if "mm2" not in stages:
                                continue
                            pend_mm2.append((kcs, pb))
                            if len(pend_mm2) > 2:
                                emit_mm2(*pend_mm2.pop(0))
                        for args in pend_mm2:
                            emit_mm2(*args)
                        if "mm2" not in stages or "norm" not in stages:
                            continue
                        # evacuate outT from PSUM quickly (frees outp for the
                        # next iteration's mm2), then run the deferred norm.
                        outsb = norm_pool.tile([65, NQ], F32, tag="outsb", name="outsb")
                        nc.vector.tensor_copy(outsb[:], outp[:])
                        for args in pended_norm:
                            norm_block(*args)
                        pended_norm = [(h, qh, outsb)]
                        if h == H - 1 and qh == 1:
                            # last iteration: run its norm eagerly (nothing
                            # left to overlap with; shortens the tail)
                            for args in pended_norm:
                                norm_block(*args)
                            pended_norm = []
        if bench_iters:
            nc.sync.dma_start(out_flag[:], o_d[0:1, 0:1])
    return nc


_CACHED = None


def _get_program():
    global _CACHED
    if _CACHED is None:
        nc = bacc.Bacc("TRN2", target_bir_lowering=False, debug=False)
        _CACHED = build_program(nc)
        _CACHED.compile()
    return _CACHED


def _make_in_maps(querys, keys, values):
    querys = np.ascontiguousarray(np.asarray(querys, dtype=np.float32))
    keys = np.ascontiguousarray(np.asarray(keys, dtype=np.float32))
    values = np.ascontiguousarray(np.asarray(values, dtype=np.float32))
    in_maps = []
    for c in range(N_CORES):
        b, hb = c // 2, c % 2
        sl = slice(hb * DD, (hb + 1) * DD)
        in_maps.append(
            {
                "querys": querys[b, :, sl],
                "keys": keys[b, :, sl],
                "values": values[b, :, sl],
            }
        )
    return in_maps


def kernel(querys, keys, values):
    nc = _get_program()
    in_maps = _make_in_maps(querys, keys, values)
    res = run_bass_kernel_spmd(nc, in_maps, list(range(N_CORES)))
    out = np.empty((B, T, U), dtype=np.float32)
    for c in range(N_CORES):
        b, hb = c // 2, c % 2
        out[b, :, hb * DD : (hb + 1) * DD] = res.results[c]["out"]
    return out
